# revision 71
# baseline (speedup 1.0000x reference)
"""Bass/Tile TRN2 kernel for nn_BertEncoder_41592463294989.

4-layer BERT encoder, KERPLE attention bias, GLU MLP.
Sharding: data-parallel over batch (B=8 -> 8 cores, 1 sequence each).

Per-core layout: activations transposed [feature, token] so every matmul
contracts over the partition dim and LayerNorm reductions (over features)
are done with ones-vector matmuls on the PE.

v2 design:
 - KERPLE bias is Toeplitz (depends only on |i-j|): exp(bias) is
   precomputed on the HOST per (layer, head) and shipped as a bf16 DRAM
   table; softmax becomes p = exp(s + padmask)*ekb. This removes all
   per-element exp/ln/pow work for the bias on the device (was 2/3 of
   ACT + half of attention DVE time).
 - All matmul operands (weights and activations) are bf16: full PE rate,
   half the weight-DMA bytes, 2x DVE rate on bf16 elementwise ops. The
   residual stream (z, h, ao) stays fp32; bf16 shadow copies feed matmuls.
 - All linear-layer biases are folded into the PE via rank-1 matmuls
   (bias row stationary, ones vector moving) instead of DVE/ACT adds.
 - V weights host-packed into per-head 65-column slots (64 features + a
   ones column) so each PV matmul also produces the softmax denominator.
 - partition broadcasts (1/denominator, LN mu/rstd) via K=1 ones-matmuls.
 - GLU and the wo projection are fused per 128-row chunk.
 - Weights packed into few DRAM tensors (dispatch cost scales with arg
   count in the PJRT path).
"""
import contextlib

import numpy as np
import ml_dtypes

import concourse.bass as bass
from concourse import bacc
import concourse.mybir as mybir
import concourse.tile as tile
from concourse.bass_utils import run_bass_kernel_spmd
from concourse.tile_rust import add_dep_helper

B, S, HID, NH, INTER, L = 8, 512, 768, 12, 3072, 4
DH = HID // NH          # 64
P = 128
NT = S // P             # 4 token tiles
KC = HID // P           # 6 hidden chunks
NIC = INTER // P        # 24 intermediate chunks
F32 = mybir.dt.float32
F32R = mybir.dt.float32r
BF16 = mybir.dt.bfloat16
NPBF16 = ml_dtypes.bfloat16
AF = mybir.ActivationFunctionType
ALU = mybir.AluOpType
HALF = NH * 65 // 2     # 390

_BUILT = {}


def _prefer_combined_act_table(arch):
    """Steer the act-table-load pass to the natural_log_exp set for exp/ln.

    The placement pass greedily first-matches each activation function
    against the table list, so alternating exp/ln picks two different
    tables and reloads on every switch. Removing exp/ln from the
    single-function sets (in the cached dict, canonical indices unchanged)
    makes both resolve to the combined set -> no reloads. The emitted
    act_func_set_id still indexes the canonical act_info.json, and the
    combined table genuinely contains both functions.
    """
    from concourse.hw_specs import get_activation_tables
    tabs = get_activation_tables(arch)
    for nm in list(tabs):
        if nm == "natural_log_exp_and_others":
            continue
        tabs[nm].discard(AF.Exp)
        tabs[nm].discard(AF.Ln)


def _layernorm(nc, tc, z_t, out_t, g_t, b_t, ones_col, ones_row, z2p, smp,
               ones_s=None, act_dep=None):
    """LN over the feature (partition x chunk) axis of z_t [P, KC, S] (F32R).

    out_t may be BF16 (mid-stack: matmul operand + residual) or F32R (last).
    If act_dep is given, a tiny dummy Exp is issued first (ordered after
    act_dep) so the natural_log_exp table load happens off the critical path.
    """
    EPS = 1e-12
    with tc.tile_pool(name="ln_ps", bufs=1, space="PSUM") as ln_ps, \
         tc.tile_pool(name="lnb_ps", bufs=1, space="PSUM") as lnb_ps:
        if act_dep is not None:
            dummy = smp.tile([1, 8], F32, tag="dummy", name="tabswitch")
            _d = nc.scalar.activation(dummy[:], ones_s[:, 0:8], AF.Exp,
                                      bias=0.0, scale=1.0)
            add_dep_helper(_d.ins, act_dep.ins, False, "act table prefetch")
        ps_sz = ln_ps.tile([1, S], F32, tag="sz")
        ps_sz2 = ln_ps.tile([1, S], F32, tag="sz2")
        for c in range(KC):
            nc.tensor.matmul(ps_sz[:], ones_col[:], z_t[:, c, :],
                             start=(c == 0), stop=(c == KC - 1))
        for c in range(KC):
            z2 = z2p.tile([P, S], F32R, tag="ztmp", name=f"zsq{c}")
            nc.scalar.activation(z2[:], z_t[:, c, :].bitcast(F32), AF.Square,
                                 bias=0.0, scale=1.0)
            nc.tensor.matmul(ps_sz2[:], ones_col[:], z2[:],
                             start=(c == 0), stop=(c == KC - 1))
        # var*H^2 = H*sz2 - sz^2 (+ eps*H^2); rstd' = rstd/H via Exp(-0.5 ln).
        # The missing 1/H on mu and H on rstd are folded into host-side
        # scaling of the LN gain (g*H) and mu (broadcast of sz/H).
        mu = smp.tile([1, S], F32R, tag="sm", name="mu")
        nc.vector.tensor_scalar(mu[:], ps_sz[:], 1.0 / HID, None, ALU.mult)
        m2h = smp.tile([1, S], F32, tag="sm", name="m2h")
        nc.vector.tensor_scalar(m2h[:], ps_sz2[:], float(HID),
                                EPS * HID * HID, ALU.mult, ALU.add)
        szsq = smp.tile([1, S], F32, tag="sm", name="szsq")
        nc.scalar.activation(szsq[:], ps_sz[:], AF.Square, bias=0.0, scale=1.0)
        var = smp.tile([1, S], F32, tag="sm", name="var")
        _var = nc.vector.tensor_tensor(var[:], m2h[:], szsq[:], ALU.subtract)
        lnv = smp.tile([1, S], F32, tag="sm", name="lnv")
        nc.scalar.activation(lnv[:], var[:], AF.Ln, bias=0.0, scale=1.0)
        rstd = smp.tile([1, S], F32R, tag="sm", name="rstd")
        nc.scalar.activation(rstd[:], lnv[:], AF.Exp, bias=0.0, scale=-0.5)
        ps_mu = lnb_ps.tile([P, S], F32, tag="mub")
        nc.tensor.matmul(ps_mu[:], ones_row[:], mu[:], start=True, stop=True)
        ps_rs = lnb_ps.tile([P, S], F32, tag="rsb")
        nc.tensor.matmul(ps_rs[:], ones_row[:], rstd[:], start=True, stop=True)
        for c in range(KC):
            t1 = z2p.tile([P, S], F32, tag="ztmp", name=f"lnt{c}")
            _s = nc.vector.tensor_tensor(t1[:], z_t[:, c, :].bitcast(F32),
                                         ps_mu[:], ALU.subtract)
            if c == 0:
                # keep the variance op ahead of the subtracts in the DVE FIFO
                add_dep_helper(_s.ins, _var.ins, False, "stats before lnt")
            # out = (t1*g)*rstd_b; the LN beta is NOT added here — it is
            # re-applied by consumers (gelu bias / residual stt scalars /
            # host-folded QKV biases), which shortens this pacing chain
            nc.vector.scalar_tensor_tensor(out_t[:, c, :], t1[:],
                                           g_t[:, c:c + 1], ps_rs[:],
                                           ALU.mult, ALU.mult)


def _build(n_layers: int):
    nc = bacc.Bacc("TRN2", target_bir_lowering=False)
    try:
        _prefer_combined_act_table(nc.m.arch)
    except Exception:
        pass

    def inp(name, shape, dt=F32):
        return nc.declare_dram_parameter(name, list(shape), dt, isOutput=False)

    # fp32 consts: mb | maskb | ones_col col | ln params | glu biases
    # (bg|bu per layer) | ones_row row | (n_layers==0 only: hT, KC*S cols)
    GB_W = 2 * L * NIC
    C32_W = (NT + S + 1 + 4 * L * KC + GB_W + P
             + (KC * S if n_layers == 0 else 0))
    c32_d = inp("c32", [P, C32_W])
    # bf16: big weight blob, per-layer layout (offsets in elements):
    #   wqk [2KC, P, KC, P] | wva [2, P, KC, HALF] | woa [KC, P, KC, P]
    #   glu [NIC, P, KC, 256] | wot [INTER, HID]
    #   bqk [2KC*P] | bva [NH*65] | boa [KC*P] | bwo [KC*P]
    # then: ones_s [S] | ekb [L, NH, P, NT*S]
    W_QKV = 2 * KC * P * KC * P
    W_V = 2 * P * KC * HALF
    W_OA = KC * P * KC * P
    W_GLU = NIC * P * KC * 256
    W_OT = INTER * HID
    W_B = 2 * KC * P + NH * 65 + KC * P + KC * P
    WLAY = W_QKV + W_V + W_OA + W_GLU + W_OT + W_B
    O_ONES_S = L * WLAY
    O_EKB = O_ONES_S + S
    w16_d = inp("w16", [O_EKB + L * NH * P * NT * S], BF16)
    h16_d = inp("h16", [HID, S], BF16)
    out_d = nc.declare_dram_parameter("out", [HID, S], F32, isOutput=True)

    def wslice(l, off, sz, shape):
        base = l * WLAY + off
        pat = " ".join(f"d{i}" for i in range(len(shape)))
        return w16_d[base:base + sz].rearrange(
            f"({pat}) -> {pat}", **{f"d{i}": shape[i] for i in range(len(shape))})

    O_QKV = 0
    O_V = O_QKV + W_QKV
    O_OA = O_V + W_V
    O_GLU = O_OA + W_OA
    O_OT = O_GLU + W_GLU
    O_BQK = O_OT + W_OT
    O_BVA = O_BQK + 2 * KC * P
    O_BOA = O_BVA + NH * 65
    O_BWO = O_BOA + KC * P

    with tile.TileContext(nc) as tc:
        lp = nc.allow_low_precision(reason="bf16 matmul operands; loose tol")
        lp.__enter__()
        stack = contextlib.ExitStack()
        const = stack.enter_context(tc.tile_pool(name="const", bufs=1))
        hpool = stack.enter_context(tc.tile_pool(name="hpool", bufs=2))
        h16p = stack.enter_context(tc.tile_pool(name="h16p", bufs=2))
        qkp = stack.enter_context(tc.tile_pool(name="qkp", bufs=1))
        vap = stack.enter_context(tc.tile_pool(name="vap", bufs=1))
        p4p = stack.enter_context(tc.tile_pool(name="p4p", bufs=3))
        ekbp = stack.enter_context(tc.tile_pool(name="ekbp", bufs=4))
        up = stack.enter_context(tc.tile_pool(name="up", bufs=2))
        atp = stack.enter_context(tc.tile_pool(name="atp", bufs=1))
        smp = stack.enter_context(tc.tile_pool(name="smp", bufs=3))
        zp = stack.enter_context(tc.tile_pool(name="zp", bufs=1))
        z2p = stack.enter_context(tc.tile_pool(name="z2p", bufs=2))
        aop = stack.enter_context(tc.tile_pool(name="aop", bufs=1))
        ao16p = stack.enter_context(tc.tile_pool(name="ao16p", bufs=1))
        xcp = stack.enter_context(tc.tile_pool(name="xcp", bufs=2))
        xgp = stack.enter_context(tc.tile_pool(name="xgp", bufs=2))
        wst = stack.enter_context(tc.tile_pool(name="wst", bufs=3))   # [128,KC,128] stream
        wvp = stack.enter_context(tc.tile_pool(name="wvp", bufs=2))   # wva halves
        wgp = stack.enter_context(tc.tile_pool(name="wgp", bufs=3))   # glu [128,KC,256]
        wop = stack.enter_context(tc.tile_pool(name="wop", bufs=3))   # wot [128,768]
        bp = stack.enter_context(tc.tile_pool(name="bp", bufs=2))
        bvp = stack.enter_context(tc.tile_pool(name="bvp", bufs=1))

        # ---- constants (h16 first: layer-0 V matmuls need it soonest) ----
        h16_t = h16p.tile([P, KC, S], BF16, tag="h16")
        nc.sync.dma_start(h16_t[:], h16_d[:].rearrange("(c p) t -> p c t", p=P))
        CW = NT + S + 1 + 4 * L * KC + GB_W
        c32_t = const.tile([P, CW], F32)
        nc.sync.dma_start(c32_t[:], c32_d[:, 0:CW])
        mb_t = c32_t[:, 0:NT]
        maskb_t = c32_t[:, NT:NT + S]
        ones_col = const.tile([P, 1], F32R)
        nc.sync.dma_start(ones_col[:],
                          c32_d[:, NT + S:NT + S + 1].bitcast(F32R))
        lnp_t = c32_t[:, NT + S + 1:]   # [P, 4*L*KC]: l1g|l1b|l2g|l2b per layer
        gb_t = c32_t[:, NT + S + 1 + 4 * L * KC:]  # [P, 2*L*NIC]: bg|bu
        ones_row = const.tile([1, P], F32R)
        nc.sync.dma_start(
            ones_row[:],
            c32_d[0:1, CW:CW + P].bitcast(F32R))
        ones_s = const.tile([1, S], BF16)
        nc.sync.dma_start(ones_s[:], w16_d[O_ONES_S:O_ONES_S + S].rearrange(
            "(a b) -> a b", a=1, b=S))

        last_gelu = [None]
        prev_exp = [None]
        for l in range(n_layers):
            ln1g_t = lnp_t[:, (4 * l) * KC:(4 * l + 1) * KC]
            ln1b_t = lnp_t[:, (4 * l + 1) * KC:(4 * l + 2) * KC]
            ln2g_t = lnp_t[:, (4 * l + 2) * KC:(4 * l + 3) * KC]
            ln2b_t = lnp_t[:, (4 * l + 3) * KC:(4 * l + 4) * KC]

            with tc.tile_pool(name="qkv_ps", bufs=2, space="PSUM") as qkv_ps, \
                 tc.tile_pool(name="sc_ps", bufs=2, space="PSUM") as sc_ps, \
                 tc.tile_pool(name="pv_ps", bufs=2, space="PSUM") as pv_ps:
                # ---------- V (token-major, head-slotted + ones col) ----------
                bva_t = bvp.tile([1, NH * 65], BF16, tag="bva")
                nc.sync.dma_start(bva_t[:], wslice(l, O_BVA, NH * 65, (1, NH * 65)))
                bqk_t = bp.tile([1, 2 * KC, P], BF16, tag="bqk")
                nc.sync.dma_start(bqk_t[:],
                                  wslice(l, O_BQK, 2 * KC * P, (1, 2 * KC, P)))
                va_t = vap.tile([P, NT, NH * 65], BF16, tag="va")
                for half in range(2):
                    sl = slice(half * HALF, (half + 1) * HALF)
                    wv = wvp.tile([P, KC, HALF], BF16, tag="wv", name=f"wv{half}")
                    nc.sync.dma_start(
                        wv[:], wslice(l, O_V + half * P * KC * HALF, P * KC * HALF,
                                      (P, KC, HALF)))
                    for jt in range(NT):
                        ps = qkv_ps.tile([P, HALF], F32, tag="qkvps",
                                         name=f"vps{half}_{jt}")
                        for kc in range(KC):
                            nc.tensor.matmul(ps[:], h16_t[:, kc, jt * P:(jt + 1) * P],
                                             wv[:, kc, :], start=(kc == 0), stop=False)
                        nc.tensor.matmul(ps[:], ones_s[:, 0:P],
                                         bva_t[:, sl], start=False, stop=True)
                        nc.scalar.activation(va_t[:, jt, sl], ps[:], AF.Copy)

                # ---------- QK + attention, interleaved per head pair ----------
                qk_t = qkp.tile([P, 2 * KC, S], BF16, tag="qk")
                at16 = atp.tile([P, KC, S], BF16, tag="attnT")

                def qk_chunk_gen(ot):
                    """One QK output chunk, yielded matmul-by-matmul so the
                    emission can be interleaved between attention matmuls of
                    the previous head pair (the PE queue is strict FIFO: QK
                    work placed ahead of dependency-stalled PV matmuls fills
                    the exp/mult latency gaps)."""
                    ps = qkv_ps.tile([P, S], F32, tag="qkvps", name=f"qk{ot}")
                    w = wst.tile([P, KC, P], BF16, tag="w", name=f"wqk{ot}")
                    nc.sync.dma_start(
                        w[:], wslice(l, O_QKV + ot * P * KC * P, P * KC * P,
                                     (P, KC, P)))
                    for kc in range(KC):
                        nc.tensor.matmul(ps[:], w[:, kc, :], h16_t[:, kc, :],
                                         start=(kc == 0), stop=False)
                        yield
                    nc.tensor.matmul(ps[:], bqk_t[:, ot, :], ones_s[:],
                                     start=False, stop=True)
                    nc.scalar.activation(qk_t[:, ot, :], ps[:], AF.Copy)
                    yield

                qk_gens = []

                def pump(n):
                    done = 0
                    while qk_gens and done < n:
                        try:
                            next(qk_gens[0])
                            done += 1
                        except StopIteration:
                            qk_gens.pop(0)

                def flush_tail(pend):
                    """Denominator tail of a finished head, deferred into the
                    next head's stream (its bc matmul's wait on rec must not
                    head-of-line-block fresh score matmuls in the PE FIFO)."""
                    ps_pv, h, rec = pend
                    ps_bc = sc_ps.tile([64, S], F32, tag="sc", name="bc")
                    nc.tensor.matmul(ps_bc[:], ones_row[:, 0:64], rec[:],
                                     start=True, stop=True)
                    rb_sb = up.tile([64, S], F32, tag="rb", name="rb_sb")
                    nc.vector.tensor_copy(rb_sb[:], ps_bc[:])
                    nc.vector.tensor_tensor(
                        at16[64 * (h % 2):64 * (h % 2) + 64, h // 2, :],
                        ps_pv[0:64, :], rb_sb[:], ALU.mult)

                for ot in (KC, 0):               # pair 0: k chunk, q chunk
                    qk_gens.append(qk_chunk_gen(ot))
                pump(1000)
                pending = None
                for hp in range(KC):
                    if hp + 1 < KC:              # stream next pair's QK
                        qk_gens.append(qk_chunk_gen(KC + hp + 1))
                        qk_gens.append(qk_chunk_gen(hp + 1))
                    for h in (2 * hp, 2 * hp + 1):
                        koff = (DH * h) % P
                        qoff = (DH * h) % P
                        ekb_t = ekbp.tile([P, NT * S], BF16, tag="ekb",
                                          name=f"ekb{h}")
                        eoff = O_EKB + (l * NH + h) * P * NT * S
                        nc.sync.dma_start(
                            ekb_t[:], w16_d[eoff:eoff + P * NT * S].rearrange(
                                "(p x) -> p x", p=P, x=NT * S))
                        p4 = p4p.tile([P, NT * S], BF16, tag="p4")
                        ps_pv = pv_ps.tile([65, S], F32, tag="pv")
                        # scores for 2 key tiles share one 2-bank psum tile so
                        # a single exp drains both (mask is folded into ekb)
                        ps_ss = []
                        for jtp in range(2):
                            ps_s2 = sc_ps.tile([P, 2 * S], F32, tag="sc",
                                               name=f"sc{jtp}")
                            ps_ss.append(ps_s2)
                            for k in range(2):
                                jt = 2 * jtp + k
                                nc.tensor.matmul(
                                    ps_s2[:, k * S:(k + 1) * S],
                                    qk_t[koff:koff + DH, KC + hp,
                                         jt * P:(jt + 1) * P],
                                    qk_t[qoff:qoff + DH, hp, :],
                                    start=True, stop=True)
                        for jtp in range(2):
                            sl = slice(2 * jtp * S, (2 * jtp + 2) * S)
                            _i = nc.scalar.activation(p4[:, sl], ps_ss[jtp][:],
                                                      AF.Exp, bias=0.0, scale=1.0)
                            if h == 0 and jtp == 0 and last_gelu[0] is not None:
                                add_dep_helper(_i.ins, last_gelu[0].ins, False,
                                               "act table grouping")
                            prev_exp[0] = _i
                            nc.vector.tensor_tensor(
                                p4[:, sl], p4[:, sl], ekb_t[:, sl], ALU.mult)
                            if jtp == 0 and pending is not None:
                                flush_tail(pending)
                                pending = None
                            pump(3)
                            for k in range(2):
                                jt = 2 * jtp + k
                                nc.tensor.matmul(
                                    ps_pv[:], va_t[:, jt, 65 * h:65 * h + 65],
                                    p4[:, jt * S:(jt + 1) * S],
                                    start=(jt == 0), stop=(jt == NT - 1))
                            pump(1)
                        rec = smp.tile([1, S], F32R, tag="sm", name="rec")
                        nc.vector.reciprocal(rec[:], ps_pv[64:65, :])
                        pending = (ps_pv, h, rec)
                    pump(1000)
                flush_tail(pending)
                pending = None

                # ---------- attention out projection + residual ----------
                boa_t = bp.tile([1, KC, P], BF16, tag="boa")
                nc.sync.dma_start(boa_t[:], wslice(l, O_BOA, KC * P, (1, KC, P)))
                z_t = zp.tile([P, KC, S], F32R, tag="z")
                for ot in range(KC):
                    ps = sc_ps.tile([P, S], F32, tag="sc", name=f"prj{ot}")
                    w = wst.tile([P, KC, P], BF16, tag="w", name=f"woa{ot}")
                    nc.sync.dma_start(
                        w[:], wslice(l, O_OA + ot * P * KC * P, P * KC * P,
                                     (P, KC, P)))
                    for kc in range(KC):
                        nc.tensor.matmul(ps[:], w[:, kc, :], at16[:, kc, :],
                                         start=(kc == 0), stop=False)
                    nc.tensor.matmul(ps[:], boa_t[:, ot, :], ones_s[:],
                                     start=False, stop=True)
                    if l == 0:
                        # layer-0 input h16 carries no deferred LN beta
                        nc.vector.tensor_tensor(z_t[:, ot, :], ps[:],
                                                h16_t[:, ot, :], ALU.add)
                    else:
                        # h16 = LN2(l-1) without beta: z = (ps + b2[l-1]) + h16
                        b2p = lnp_t[:, (4 * (l - 1) + 3) * KC + ot:
                                    (4 * (l - 1) + 3) * KC + ot + 1]
                        nc.vector.scalar_tensor_tensor(
                            z_t[:, ot, :], ps[:], b2p, h16_t[:, ot, :],
                            ALU.add, ALU.add)

            # ---------- LN1 ----------
            ao16 = ao16p.tile([P, KC, S], BF16, tag="ao16")
            _layernorm(nc, tc, z_t, ao16, ln1g_t, ln1b_t, ones_col,
                       ones_row, z2p, smp)

            # ---------- GLU + wo (fused) ----------
            with tc.tile_pool(name="glu_ps", bufs=1, space="PSUM") as glu_ps, \
                 tc.tile_pool(name="wo_ps", bufs=6, space="PSUM") as wo_ps:
                bwo_t = bp.tile([1, KC, P], BF16, tag="bwo")
                nc.sync.dma_start(bwo_t[:], wslice(l, O_BWO, KC * P, (1, KC, P)))

                wo_acc = [wo_ps.tile([P, S], F32, tag="woacc", name=f"woacc{i}")
                          for i in range(KC)]
                for gt in range(NIC):
                    ps_g = glu_ps.tile([P, S], F32, tag="gps")
                    ps_u = glu_ps.tile([P, S], F32, tag="ups")
                    gw = wgp.tile([P, KC, 256], BF16, tag="gw", name=f"gw{gt}")
                    nc.sync.dma_start(
                        gw[:], wslice(l, O_GLU + gt * P * KC * 256, P * KC * 256,
                                      (P, KC, 256)))
                    for kc in range(KC):
                        nc.tensor.matmul(ps_g[:], gw[:, kc, 0:128], ao16[:, kc, :],
                                         start=(kc == 0), stop=(kc == KC - 1))
                    for kc in range(KC):
                        nc.tensor.matmul(ps_u[:], gw[:, kc, 128:256], ao16[:, kc, :],
                                         start=(kc == 0), stop=(kc == KC - 1))
                    xg = xgp.tile([P, S], BF16, tag="xg")
                    # gelu bias = glu_w[:INTER] @ ln1_b (deferred LN1 beta)
                    _i = nc.scalar.activation(
                        xg[:], ps_g[:], AF.Gelu,
                        bias=gb_t[:, 2 * l * NIC + gt:2 * l * NIC + gt + 1],
                        scale=1.0)
                    if gt == 0 and prev_exp[0] is not None:
                        add_dep_helper(_i.ins, prev_exp[0].ins, False,
                                       "act table grouping")
                    last_gelu[0] = _i
                    xc = xcp.tile([P, S], BF16, tag="xc")
                    # xc = (ps_u + bu)*xg, bu = glu_w[INTER:] @ ln1_b
                    nc.vector.scalar_tensor_tensor(
                        xc[:], ps_u[:],
                        gb_t[:, (2 * l + 1) * NIC + gt:(2 * l + 1) * NIC + gt + 1],
                        xg[:], ALU.add, ALU.mult)
                    wot_t = wop.tile([P, HID], BF16, tag="wot")
                    nc.sync.dma_start(
                        wot_t[:], wslice(l, O_OT + gt * P * HID, P * HID, (P, HID)))
                    for ot in range(KC):
                        nc.tensor.matmul(wo_acc[ot][:], wot_t[:, ot * P:(ot + 1) * P],
                                         xc[:], start=(gt == 0), stop=False)
                z2_t = zp.tile([P, KC, S], F32R, tag="z", name="z_mlp")
                for ot in range(KC):
                    nc.tensor.matmul(wo_acc[ot][:], bwo_t[:, ot, :], ones_s[:],
                                     start=False, stop=True)
                    # ao16 = LN1 without beta: z2 = (wo + b1) + ao16
                    b1p = lnp_t[:, (4 * l + 1) * KC + ot:
                                (4 * l + 1) * KC + ot + 1]
                    nc.vector.scalar_tensor_tensor(
                        z2_t[:, ot, :], wo_acc[ot][:], b1p, ao16[:, ot, :],
                        ALU.add, ALU.add)

            # ---------- LN2 -> next h ----------
            if l + 1 < n_layers:
                h16_t = h16p.tile([P, KC, S], BF16, tag="h16",
                                  name=f"h16_{l + 1}")
                _layernorm(nc, tc, z2_t, h16_t, ln2g_t, ln2b_t, ones_col,
                           ones_row, z2p, smp, ones_s, last_gelu[0])
            else:
                h_t = hpool.tile([P, KC, S], F32R, tag="h", name="h_last")
                _layernorm(nc, tc, z2_t, h_t, ln2g_t, ln2b_t, ones_col,
                           ones_row, z2p, smp, ones_s, last_gelu[0])

        # ---------- final mask + store ----------
        out_sb = zp.tile([P, KC, S], F32, tag="z", name="out_sb")
        if n_layers == 0:
            h_t = hpool.tile([P, KC, S], F32R, tag="h", name="h_last")
            hoff = NT + S + 1 + 4 * L * KC + GB_W + P
            nc.sync.dma_start(
                h_t[:], c32_d[:, hoff:hoff + KC * S].rearrange(
                    "p (c t) -> p c t", c=KC, t=S).bitcast(F32R))
        out_view = out_d[:].rearrange("(c p) t -> p c t", p=P)
        for c in range(KC):
            if n_layers == 0:
                nc.vector.tensor_tensor(out_sb[:, c, :],
                                        h_t[:, c, :].bitcast(F32),
                                        maskb_t[:], ALU.mult)
            else:
                # h = LN2(last) without beta: out = (h + b2)*mask
                b2p = lnp_t[:, (4 * (n_layers - 1) + 3) * KC + c:
                            (4 * (n_layers - 1) + 3) * KC + c + 1]
                nc.vector.scalar_tensor_tensor(
                    out_sb[:, c, :], h_t[:, c, :].bitcast(F32), b2p,
                    maskb_t[:], ALU.add, ALU.mult)
            nc.sync.dma_start(out_view[:, c, :], out_sb[:, c, :])

        stack.close()
        lp.__exit__(None, None, None)

    nc.finalize()
    return nc


def _prep_inputs(hidden_states, attention_mask, Wqkv_w, Wqkv_b, attn_out_w,
                 attn_out_b, ln1_g, ln1_b, glu_w, wo_w, wo_b, ln2_g, ln2_b,
                 r1, r2, r3):
    """Host-side sharding + weight layout transforms (shared across cores)."""
    f32 = np.float32
    shared = {}

    # ekb: exp(kerple bias) per (layer, head), Toeplitz [S, S] -> [P, NT*S]
    c1 = np.clip(r1.reshape(L, NH).astype(np.float64), 1e-7, None)
    c2 = np.clip(r2.reshape(L, NH).astype(np.float64), 1e-7, None)
    c3 = np.clip(r3.reshape(L, NH).astype(np.float64), 1e-7, None)
    idx = np.arange(S)
    rel = np.abs(idx[None, :] - idx[:, None]).astype(np.float64)  # [j, i]
    ekb = np.empty((L, NH, P, NT, S), f32)
    for l in range(L):
        for h in range(NH):
            relp = np.where(rel > 0, rel, 1.0) ** c3[l, h]
            relp = np.where(rel > 0, relp, 0.0)
            m = np.exp(-c1[l, h] * np.log1p(c2[l, h] * relp))  # [j, i]
            # [j, i] -> [jt, p, i] -> [p, jt, i]
            ekb[l, h] = m.reshape(NT, P, S).transpose(1, 0, 2).astype(f32)

    # LN outputs are stored WITHOUT beta on-device; fold each layer's
    # consumed beta into the next linear layer's bias: for layer l>=1 the
    # QKV input is LN2(l-1) missing ln2_b[l-1].
    Wqkv_b = np.asarray(Wqkv_b, f32).copy()
    for l in range(1, L):
        Wqkv_b[l] = Wqkv_b[l] + np.asarray(Wqkv_w[l], f32) @ np.asarray(
            ln2_b[l - 1], f32)

    wq = Wqkv_w[:, :HID, :] / 8.0           # fold 1/sqrt(DH)
    wk = Wqkv_w[:, HID:2 * HID, :]
    bq = Wqkv_b[:, :HID] / 8.0
    bk = Wqkv_b[:, HID:2 * HID]
    wqk = np.concatenate([wq, wk], axis=1)  # [L, 1536, HID]
    wqkT = np.transpose(wqk, (0, 2, 1))     # [L, HID, 1536]
    wqk_p = np.ascontiguousarray(
        wqkT.reshape(L, KC, P, 2 * KC, P).transpose(0, 3, 2, 1, 4))
    bqk_p = np.concatenate([bq, bk], axis=1)  # [L, 1536]

    wv = Wqkv_w[:, 2 * HID:, :]             # [L, 768v, 768]
    bv = Wqkv_b[:, 2 * HID:]
    wva = np.zeros((L, HID, NH * 65), f32)
    bva_p = np.zeros((L, NH * 65), f32)
    for h in range(NH):
        wva[:, :, 65 * h:65 * h + 64] = np.transpose(
            wv[:, DH * h:DH * (h + 1), :], (0, 2, 1))
        bva_p[:, 65 * h:65 * h + 64] = bv[:, DH * h:DH * (h + 1)]
        bva_p[:, 65 * h + 64] = 1.0
    wva_p = np.ascontiguousarray(
        wva.reshape(L, KC, P, 2, HALF).transpose(0, 3, 2, 1, 4))

    woaT = np.transpose(attn_out_w, (0, 2, 1))  # [L, HID, HID]
    woa_p = np.ascontiguousarray(
        woaT.reshape(L, KC, P, KC, P).transpose(0, 3, 2, 1, 4))

    glu = np.empty((L, HID, NIC, 256), f32)
    gw = np.transpose(glu_w, (0, 2, 1))     # [L, HID, 6144]
    for gt in range(NIC):
        glu[:, :, gt, 0:128] = gw[:, :, gt * P:(gt + 1) * P]
        glu[:, :, gt, 128:256] = gw[:, :, INTER + gt * P:INTER + (gt + 1) * P]
    glu_p = np.ascontiguousarray(
        glu.reshape(L, KC, P, NIC, 256).transpose(0, 3, 2, 1, 4))
    wot_p = np.ascontiguousarray(np.transpose(wo_w, (0, 2, 1)))  # [L, INTER, HID]

    w16 = np.concatenate([
        wqk_p.reshape(L, -1), wva_p.reshape(L, -1), woa_p.reshape(L, -1),
        glu_p.reshape(L, -1), wot_p.reshape(L, -1),
        bqk_p.reshape(L, -1), bva_p.reshape(L, -1),
        attn_out_b.reshape(L, -1), wo_b.reshape(L, -1),
    ], axis=1).astype(NPBF16)
    w16_head = np.concatenate([w16.reshape(-1), np.ones(S, NPBF16)])

    def pcol(v):  # [L, 768] -> [L, P, KC]
        return np.ascontiguousarray(v.reshape(L, KC, P).transpose(0, 2, 1)).astype(f32)

    # gains scaled by HID: the kernel computes rstd/HID (see _layernorm)
    lnp = np.stack([pcol(ln1_g * HID), pcol(ln1_b), pcol(ln2_g * HID),
                    pcol(ln2_b)], axis=1)  # [L, 4, P, KC]
    lnp = np.ascontiguousarray(lnp.transpose(2, 0, 1, 3)).reshape(P, 4 * L * KC)
    # deferred-LN1-beta glu biases: bg = glu_w[:,:INTER]... wait: glu rows are
    # outputs; bg[l] = glu_w[l, :INTER] @ ln1_b[l], bu[l] = glu_w[l, INTER:] @ b
    gbias = np.einsum("lof,lf->lo", np.asarray(glu_w, f32),
                      np.asarray(ln1_b, f32))      # [L, 2*INTER]
    bg = gbias[:, :INTER].reshape(L, NIC, P)       # [L, gt, p]
    bu = gbias[:, INTER:].reshape(L, NIC, P)
    gb = np.concatenate([bg, bu], axis=1)          # [L, 2*NIC, P]
    gb = np.ascontiguousarray(gb.transpose(2, 0, 1)).reshape(P, 2 * L * NIC)

    n_layers = _prep_inputs._n_layers
    in_maps = []
    for b in range(B):
        m = dict(shared)
        hTb = np.ascontiguousarray(hidden_states[b].T).astype(f32)
        m["h16"] = hTb.astype(NPBF16)
        mask = attention_mask[b].astype(f32)          # [S]
        # fold the key-side padding mask into the kerple table: softmax
        # numerator is exp(s)*ekb*mask_j, exactly 0 at padded keys
        mfold = mask.reshape(NT, P).T                 # [P, NT]
        ekb_b = (ekb * mfold[None, None, :, :, None]).astype(NPBF16)
        m["w16"] = np.ascontiguousarray(
            np.concatenate([w16_head, ekb_b.reshape(-1)]))
        mbias = (1.0 - mask) * -10000.0
        gbw = 2 * L * NIC
        base = NT + S + 1 + 4 * L * KC + gbw
        c32 = np.zeros((P, base + P + (KC * S if n_layers == 0 else 0)), f32)
        c32[:, 0:NT] = mbias.reshape(NT, P).T
        c32[:, NT:NT + S] = mask[None, :]
        c32[:, NT + S] = 1.0                      # ones_col
        c32[:, NT + S + 1:base - gbw] = lnp
        c32[:, base - gbw:base] = gb
        c32[0, base:base + P] = 1.0               # ones_row
        if n_layers == 0:
            c32[:, base + P:] = hTb.reshape(KC, P, S).transpose(1, 0, 2
                                                               ).reshape(P, KC * S)
        m["c32"] = c32
        in_maps.append(m)
    return in_maps


_prep_inputs._n_layers = L


def kernel(**inputs) -> np.ndarray:
    n_layers = int(inputs.pop("_n_layers", L))
    if n_layers not in _BUILT:
        _BUILT[n_layers] = _build(n_layers)
    nc = _BUILT[n_layers]
    _prep_inputs._n_layers = n_layers
    in_maps = _prep_inputs(**inputs)
    res = run_bass_kernel_spmd(nc, in_maps, list(range(B))).results
    out = np.empty((B, S, HID), np.float32)
    for b in range(B):
        out[b] = res[b]["out"].T
    return out


# revision 73
# speedup vs baseline: 1.1017x; 1.1017x over previous
"""Bass/Tile TRN2 kernel for nn_BertEncoder_41592463294989.

4-layer BERT encoder, KERPLE attention bias, GLU MLP.
Sharding: data-parallel over batch (B=8 -> 8 cores, 1 sequence each).

Per-core layout: activations transposed [feature, token] so every matmul
contracts over the partition dim and LayerNorm reductions (over features)
are done with ones-vector matmuls on the PE.

v2 design:
 - KERPLE bias is Toeplitz (depends only on |i-j|): exp(bias) is
   precomputed on the HOST per (layer, head) and shipped as a bf16 DRAM
   table; softmax becomes p = exp(s + padmask)*ekb. This removes all
   per-element exp/ln/pow work for the bias on the device (was 2/3 of
   ACT + half of attention DVE time).
 - All matmul operands (weights and activations) are bf16: full PE rate,
   half the weight-DMA bytes, 2x DVE rate on bf16 elementwise ops. The
   residual stream (z, h, ao) stays fp32; bf16 shadow copies feed matmuls.
 - All linear-layer biases are folded into the PE via rank-1 matmuls
   (bias row stationary, ones vector moving) instead of DVE/ACT adds.
 - V weights host-packed into per-head 65-column slots (64 features + a
   ones column) so each PV matmul also produces the softmax denominator.
 - partition broadcasts (1/denominator, LN mu/rstd) via K=1 ones-matmuls.
 - GLU and the wo projection are fused per 128-row chunk.
 - Weights packed into few DRAM tensors (dispatch cost scales with arg
   count in the PJRT path).
"""
import contextlib

import numpy as np
import ml_dtypes

import concourse.bass as bass
from concourse import bacc
import concourse.mybir as mybir
import concourse.tile as tile
from concourse.bass_utils import run_bass_kernel_spmd
from concourse.tile_rust import add_dep_helper

B, S, HID, NH, INTER, L = 8, 512, 768, 12, 3072, 4
DH = HID // NH          # 64
P = 128
NT = S // P             # 4 token tiles
KC = HID // P           # 6 hidden chunks
NIC = INTER // P        # 24 intermediate chunks
F32 = mybir.dt.float32
F32R = mybir.dt.float32r
BF16 = mybir.dt.bfloat16
NPBF16 = ml_dtypes.bfloat16
AF = mybir.ActivationFunctionType
ALU = mybir.AluOpType
HALF = NH * 65 // 2     # 390

_BUILT = {}


def _prefer_combined_act_table(arch):
    """Steer the act-table-load pass to the natural_log_exp set for exp/ln.

    The placement pass greedily first-matches each activation function
    against the table list, so alternating exp/ln picks two different
    tables and reloads on every switch. Removing exp/ln from the
    single-function sets (in the cached dict, canonical indices unchanged)
    makes both resolve to the combined set -> no reloads. The emitted
    act_func_set_id still indexes the canonical act_info.json, and the
    combined table genuinely contains both functions.
    """
    from concourse.hw_specs import get_activation_tables
    tabs = get_activation_tables(arch)
    for nm in list(tabs):
        if nm == "natural_log_exp_and_others":
            continue
        tabs[nm].discard(AF.Exp)
        tabs[nm].discard(AF.Ln)


def _layernorm(nc, tc, z_t, out_t, g_t, b_t, ones_col, ones_row, z2p, smp,
               ones_s=None, act_dep=None):
    """LN over the feature (partition x chunk) axis of z_t [P, KC, S] (F32R).

    out_t may be BF16 (mid-stack: matmul operand + residual) or F32R (last).
    If act_dep is given, a tiny dummy Exp is issued first (ordered after
    act_dep) so the natural_log_exp table load happens off the critical path.
    """
    EPS = 1e-12
    with tc.tile_pool(name="ln_ps", bufs=1, space="PSUM") as ln_ps, \
         tc.tile_pool(name="lnb_ps", bufs=1, space="PSUM") as lnb_ps:
        if act_dep is not None:
            dummy = smp.tile([1, 8], F32, tag="dummy", name="tabswitch")
            _d = nc.scalar.activation(dummy[:], ones_s[:, 0:8], AF.Exp,
                                      bias=0.0, scale=1.0)
            add_dep_helper(_d.ins, act_dep.ins, False, "act table prefetch")
        ps_sz = ln_ps.tile([1, S], F32, tag="sz")
        ps_sz2 = ln_ps.tile([1, S], F32, tag="sz2")
        for c in range(KC):
            nc.tensor.matmul(ps_sz[:], ones_col[:], z_t[:, c, :],
                             start=(c == 0), stop=(c == KC - 1))
        for c in range(KC):
            z2 = z2p.tile([P, S], F32R, tag="ztmp", name=f"zsq{c}")
            nc.scalar.activation(z2[:], z_t[:, c, :].bitcast(F32), AF.Square,
                                 bias=0.0, scale=1.0)
            nc.tensor.matmul(ps_sz2[:], ones_col[:], z2[:],
                             start=(c == 0), stop=(c == KC - 1))
        # var*H^2 = H*sz2 - sz^2 (+ eps*H^2); rstd' = rstd/H via Exp(-0.5 ln).
        # The missing 1/H on mu and H on rstd are folded into host-side
        # scaling of the LN gain (g*H) and mu (broadcast of sz/H).
        mu = smp.tile([1, S], F32R, tag="sm", name="mu")
        nc.vector.tensor_scalar(mu[:], ps_sz[:], 1.0 / HID, None, ALU.mult)
        m2h = smp.tile([1, S], F32, tag="sm", name="m2h")
        nc.vector.tensor_scalar(m2h[:], ps_sz2[:], float(HID),
                                EPS * HID * HID, ALU.mult, ALU.add)
        szsq = smp.tile([1, S], F32, tag="sm", name="szsq")
        nc.scalar.activation(szsq[:], ps_sz[:], AF.Square, bias=0.0, scale=1.0)
        var = smp.tile([1, S], F32, tag="sm", name="var")
        _var = nc.vector.tensor_tensor(var[:], m2h[:], szsq[:], ALU.subtract)
        lnv = smp.tile([1, S], F32, tag="sm", name="lnv")
        nc.scalar.activation(lnv[:], var[:], AF.Ln, bias=0.0, scale=1.0)
        rstd = smp.tile([1, S], F32R, tag="sm", name="rstd")
        nc.scalar.activation(rstd[:], lnv[:], AF.Exp, bias=0.0, scale=-0.5)
        ps_mu = lnb_ps.tile([P, S], F32, tag="mub")
        nc.tensor.matmul(ps_mu[:], ones_row[:], mu[:], start=True, stop=True)
        ps_rs = lnb_ps.tile([P, S], F32, tag="rsb")
        nc.tensor.matmul(ps_rs[:], ones_row[:], rstd[:], start=True, stop=True)
        for c in range(KC):
            t1 = z2p.tile([P, S], F32, tag="ztmp", name=f"lnt{c}")
            _s = nc.vector.tensor_tensor(t1[:], z_t[:, c, :].bitcast(F32),
                                         ps_mu[:], ALU.subtract)
            if c == 0:
                # keep the variance op ahead of the subtracts in the DVE FIFO
                add_dep_helper(_s.ins, _var.ins, False, "stats before lnt")
            # out = (t1*g)*rstd_b; the LN beta is NOT added here — it is
            # re-applied by consumers (gelu bias / residual stt scalars /
            # host-folded QKV biases), which shortens this pacing chain
            nc.vector.scalar_tensor_tensor(out_t[:, c, :], t1[:],
                                           g_t[:, c:c + 1], ps_rs[:],
                                           ALU.mult, ALU.mult)


def _build(n_layers: int):
    nc = bacc.Bacc("TRN2", target_bir_lowering=False)
    try:
        _prefer_combined_act_table(nc.m.arch)
    except Exception:
        pass

    def inp(name, shape, dt=F32):
        return nc.declare_dram_parameter(name, list(shape), dt, isOutput=False)

    # fp32 consts: mb | maskb | ones_col col | ln params | glu biases
    # (bg|bu per layer) | ones_row row | (n_layers==0 only: hT, KC*S cols)
    GB_W = 2 * L * NIC
    C32_W = (NT + S + 1 + 4 * L * KC + GB_W + P
             + (KC * S if n_layers == 0 else 0))
    c32_d = inp("c32", [P, C32_W])
    # bf16: big weight blob, per-layer layout (offsets in elements):
    #   wqk [2KC, P, KC, P] | wva [2, P, KC, HALF] | woa [KC, P, KC, P]
    #   glu [NIC, P, KC, 256] | wot [INTER, HID]
    #   bqk [2KC*P] | bva [NH*65] | boa [KC*P] | bwo [KC*P]
    # then: ones_s [S] | ekb [L, NH, P, NT*S]
    W_QKV = 2 * KC * P * KC * P
    W_V = 2 * P * KC * HALF
    W_OA = KC * P * KC * P
    W_GLU = NIC * P * KC * 256
    W_OT = INTER * HID
    W_B = 2 * KC * P + NH * 65 + KC * P + KC * P
    WLAY = W_QKV + W_V + W_OA + W_GLU + W_OT + W_B
    O_ONES_S = L * WLAY
    O_EKB = O_ONES_S + S
    w16_d = inp("w16", [O_EKB + L * NH * P * NT * S], BF16)
    h16_d = inp("h16", [HID, S], BF16)
    out_d = nc.declare_dram_parameter("out", [HID, S], F32, isOutput=True)

    def wslice(l, off, sz, shape):
        base = l * WLAY + off
        pat = " ".join(f"d{i}" for i in range(len(shape)))
        return w16_d[base:base + sz].rearrange(
            f"({pat}) -> {pat}", **{f"d{i}": shape[i] for i in range(len(shape))})

    O_QKV = 0
    O_V = O_QKV + W_QKV
    O_OA = O_V + W_V
    O_GLU = O_OA + W_OA
    O_OT = O_GLU + W_GLU
    O_BQK = O_OT + W_OT
    O_BVA = O_BQK + 2 * KC * P
    O_BOA = O_BVA + NH * 65
    O_BWO = O_BOA + KC * P

    with tile.TileContext(nc) as tc:
        lp = nc.allow_low_precision(reason="bf16 matmul operands; loose tol")
        lp.__enter__()
        stack = contextlib.ExitStack()
        const = stack.enter_context(tc.tile_pool(name="const", bufs=1))
        hpool = stack.enter_context(tc.tile_pool(name="hpool", bufs=2))
        h16p = stack.enter_context(tc.tile_pool(name="h16p", bufs=2))
        qkp = stack.enter_context(tc.tile_pool(name="qkp", bufs=1))
        vap = stack.enter_context(tc.tile_pool(name="vap", bufs=1))
        p4p = stack.enter_context(tc.tile_pool(name="p4p", bufs=3))
        ekbp = stack.enter_context(tc.tile_pool(name="ekbp", bufs=4))
        up = stack.enter_context(tc.tile_pool(name="up", bufs=2))
        atp = stack.enter_context(tc.tile_pool(name="atp", bufs=1))
        smp = stack.enter_context(tc.tile_pool(name="smp", bufs=3))
        zp = stack.enter_context(tc.tile_pool(name="zp", bufs=1))
        z2p = stack.enter_context(tc.tile_pool(name="z2p", bufs=2))
        aop = stack.enter_context(tc.tile_pool(name="aop", bufs=1))
        ao16p = stack.enter_context(tc.tile_pool(name="ao16p", bufs=1))
        xcp = stack.enter_context(tc.tile_pool(name="xcp", bufs=2))
        xgp = stack.enter_context(tc.tile_pool(name="xgp", bufs=2))
        wst = stack.enter_context(tc.tile_pool(name="wst", bufs=3))   # [128,KC,128] stream
        wvp = stack.enter_context(tc.tile_pool(name="wvp", bufs=2))   # wva halves
        wgp = stack.enter_context(tc.tile_pool(name="wgp", bufs=3))   # glu [128,KC,256]
        wop = stack.enter_context(tc.tile_pool(name="wop", bufs=3))   # wot [128,768]
        bp = stack.enter_context(tc.tile_pool(name="bp", bufs=2))
        bvp = stack.enter_context(tc.tile_pool(name="bvp", bufs=1))

        # ---- constants (h16 first: layer-0 V matmuls need it soonest) ----
        h16_t = h16p.tile([P, KC, S], BF16, tag="h16")
        nc.sync.dma_start(h16_t[:], h16_d[:].rearrange("(c p) t -> p c t", p=P))
        CW = NT + S + 1 + 4 * L * KC + GB_W
        c32_t = const.tile([P, CW], F32)
        nc.sync.dma_start(c32_t[:], c32_d[:, 0:CW])
        mb_t = c32_t[:, 0:NT]
        maskb_t = c32_t[:, NT:NT + S]
        ones_col = const.tile([P, 1], F32R)
        nc.sync.dma_start(ones_col[:],
                          c32_d[:, NT + S:NT + S + 1].bitcast(F32R))
        lnp_t = c32_t[:, NT + S + 1:]   # [P, 4*L*KC]: l1g|l1b|l2g|l2b per layer
        gb_t = c32_t[:, NT + S + 1 + 4 * L * KC:]  # [P, 2*L*NIC]: bg|bu
        ones_row = const.tile([1, P], F32R)
        nc.sync.dma_start(
            ones_row[:],
            c32_d[0:1, CW:CW + P].bitcast(F32R))
        ones_s = const.tile([1, S], BF16)
        nc.sync.dma_start(ones_s[:], w16_d[O_ONES_S:O_ONES_S + S].rearrange(
            "(a b) -> a b", a=1, b=S))

        last_gelu = [None]
        prev_exp = [None]
        for l in range(n_layers):
            ln1g_t = lnp_t[:, (4 * l) * KC:(4 * l + 1) * KC]
            ln1b_t = lnp_t[:, (4 * l + 1) * KC:(4 * l + 2) * KC]
            ln2g_t = lnp_t[:, (4 * l + 2) * KC:(4 * l + 3) * KC]
            ln2b_t = lnp_t[:, (4 * l + 3) * KC:(4 * l + 4) * KC]

            with tc.tile_pool(name="qkv_ps", bufs=2, space="PSUM") as qkv_ps, \
                 tc.tile_pool(name="sc_ps", bufs=2, space="PSUM") as sc_ps, \
                 tc.tile_pool(name="pv_ps", bufs=2, space="PSUM") as pv_ps:
                # ---------- V (token-major, head-slotted + ones col) ----------
                bva_t = bvp.tile([1, NH * 65], BF16, tag="bva")
                nc.sync.dma_start(bva_t[:], wslice(l, O_BVA, NH * 65, (1, NH * 65)))
                bqk_t = bp.tile([1, 2 * KC, P], BF16, tag="bqk")
                nc.sync.dma_start(bqk_t[:],
                                  wslice(l, O_BQK, 2 * KC * P, (1, 2 * KC, P)))
                va_t = vap.tile([P, NT, NH * 65], BF16, tag="va")
                for half in range(2):
                    sl = slice(half * HALF, (half + 1) * HALF)
                    wv = wvp.tile([P, KC, HALF], BF16, tag="wv", name=f"wv{half}")
                    nc.sync.dma_start(
                        wv[:], wslice(l, O_V + half * P * KC * HALF, P * KC * HALF,
                                      (P, KC, HALF)))
                    for jt in range(NT):
                        vpool = qkv_ps if jt % 2 == 0 else sc_ps
                        ps = vpool.tile([P, HALF], F32,
                                        tag="qkvps" if jt % 2 == 0 else "sc",
                                        name=f"vps{half}_{jt}")
                        for kc in range(KC):
                            nc.tensor.matmul(ps[:], h16_t[:, kc, jt * P:(jt + 1) * P],
                                             wv[:, kc, :], start=(kc == 0), stop=False)
                        nc.tensor.matmul(ps[:], ones_s[:, 0:P],
                                         bva_t[:, sl], start=False, stop=True)
                        nc.scalar.activation(va_t[:, jt, sl], ps[:], AF.Copy)

                # ---------- QK + attention, interleaved per head pair ----------
                qk_t = qkp.tile([P, 2 * KC, S], BF16, tag="qk")
                at16 = atp.tile([P, KC, S], BF16, tag="attnT")

                def qk_chunk_gen(ot):
                    """One QK output chunk, yielded matmul-by-matmul so the
                    emission can be interleaved between attention matmuls of
                    the previous head pair (the PE queue is strict FIFO: QK
                    work placed ahead of dependency-stalled PV matmuls fills
                    the exp/mult latency gaps)."""
                    ps = qkv_ps.tile([P, S], F32, tag="qkvps", name=f"qk{ot}")
                    w = wst.tile([P, KC, P], BF16, tag="w", name=f"wqk{ot}")
                    nc.sync.dma_start(
                        w[:], wslice(l, O_QKV + ot * P * KC * P, P * KC * P,
                                     (P, KC, P)))
                    for kc in range(KC):
                        nc.tensor.matmul(ps[:], w[:, kc, :], h16_t[:, kc, :],
                                         start=(kc == 0), stop=False)
                        yield
                    nc.tensor.matmul(ps[:], bqk_t[:, ot, :], ones_s[:],
                                     start=False, stop=True)
                    nc.scalar.activation(qk_t[:, ot, :], ps[:], AF.Copy)
                    yield

                qk_gens = []

                def pump(n):
                    done = 0
                    while qk_gens and done < n:
                        try:
                            next(qk_gens[0])
                            done += 1
                        except StopIteration:
                            qk_gens.pop(0)

                def flush_tail(pend):
                    """Denominator tail of a finished head, deferred into the
                    next head's stream (its bc matmul's wait on rec must not
                    head-of-line-block fresh score matmuls in the PE FIFO)."""
                    ps_pv, h, rec = pend
                    ps_bc = sc_ps.tile([64, S], F32, tag="sc", name="bc")
                    nc.tensor.matmul(ps_bc[:], ones_row[:, 0:64], rec[:],
                                     start=True, stop=True)
                    rb_sb = up.tile([64, S], F32, tag="rb", name="rb_sb")
                    nc.vector.tensor_copy(rb_sb[:], ps_bc[:])
                    nc.vector.tensor_tensor(
                        at16[64 * (h % 2):64 * (h % 2) + 64, h // 2, :],
                        ps_pv[0:64, :], rb_sb[:], ALU.mult)

                for ot in (KC, 0):               # pair 0: k chunk, q chunk
                    qk_gens.append(qk_chunk_gen(ot))
                pump(1000)
                pending = None
                for hp in range(KC):
                    if hp + 1 < KC:              # stream next pair's QK
                        qk_gens.append(qk_chunk_gen(KC + hp + 1))
                        qk_gens.append(qk_chunk_gen(hp + 1))
                    for h in (2 * hp, 2 * hp + 1):
                        koff = (DH * h) % P
                        qoff = (DH * h) % P
                        ekb_t = ekbp.tile([P, NT * S], BF16, tag="ekb",
                                          name=f"ekb{h}")
                        eoff = O_EKB + (l * NH + h) * P * NT * S
                        nc.sync.dma_start(
                            ekb_t[:], w16_d[eoff:eoff + P * NT * S].rearrange(
                                "(p x) -> p x", p=P, x=NT * S))
                        p4 = p4p.tile([P, NT * S], BF16, tag="p4")
                        ps_pv = pv_ps.tile([65, S], F32, tag="pv")
                        # scores for 2 key tiles share one 2-bank psum tile so
                        # a single exp drains both (mask is folded into ekb)
                        ps_ss = []
                        for jtp in range(2):
                            ps_s2 = sc_ps.tile([P, 2 * S], F32, tag="sc",
                                               name=f"sc{jtp}")
                            ps_ss.append(ps_s2)
                            for k in range(2):
                                jt = 2 * jtp + k
                                nc.tensor.matmul(
                                    ps_s2[:, k * S:(k + 1) * S],
                                    qk_t[koff:koff + DH, KC + hp,
                                         jt * P:(jt + 1) * P],
                                    qk_t[qoff:qoff + DH, hp, :],
                                    start=True, stop=True)
                        for jtp in range(2):
                            sl = slice(2 * jtp * S, (2 * jtp + 2) * S)
                            _i = nc.scalar.activation(p4[:, sl], ps_ss[jtp][:],
                                                      AF.Exp, bias=0.0, scale=1.0)
                            if h == 0 and jtp == 0 and last_gelu[0] is not None:
                                add_dep_helper(_i.ins, last_gelu[0].ins, False,
                                               "act table grouping")
                            prev_exp[0] = _i
                            nc.vector.tensor_tensor(
                                p4[:, sl], p4[:, sl], ekb_t[:, sl], ALU.mult)
                            if jtp == 0 and pending is not None:
                                flush_tail(pending)
                                pending = None
                            pump(3)
                            for k in range(2):
                                jt = 2 * jtp + k
                                nc.tensor.matmul(
                                    ps_pv[:], va_t[:, jt, 65 * h:65 * h + 65],
                                    p4[:, jt * S:(jt + 1) * S],
                                    start=(jt == 0), stop=(jt == NT - 1))
                            pump(1)
                        rec = smp.tile([1, S], F32R, tag="sm", name="rec")
                        nc.vector.reciprocal(rec[:], ps_pv[64:65, :])
                        pending = (ps_pv, h, rec)
                    pump(1000)
                flush_tail(pending)
                pending = None

                # ---------- attention out projection + residual ----------
                boa_t = bp.tile([1, KC, P], BF16, tag="boa")
                nc.sync.dma_start(boa_t[:], wslice(l, O_BOA, KC * P, (1, KC, P)))
                z_t = zp.tile([P, KC, S], F32R, tag="z")
                for ot in range(KC):
                    ppool = sc_ps if ot % 2 == 0 else qkv_ps
                    ps = ppool.tile([P, S], F32,
                                    tag="sc" if ot % 2 == 0 else "qkvps",
                                    name=f"prj{ot}")
                    w = wst.tile([P, KC, P], BF16, tag="w", name=f"woa{ot}")
                    nc.sync.dma_start(
                        w[:], wslice(l, O_OA + ot * P * KC * P, P * KC * P,
                                     (P, KC, P)))
                    for kc in range(KC):
                        nc.tensor.matmul(ps[:], w[:, kc, :], at16[:, kc, :],
                                         start=(kc == 0), stop=False)
                    nc.tensor.matmul(ps[:], boa_t[:, ot, :], ones_s[:],
                                     start=False, stop=True)
                    if l == 0:
                        # layer-0 input h16 carries no deferred LN beta
                        nc.vector.tensor_tensor(z_t[:, ot, :], ps[:],
                                                h16_t[:, ot, :], ALU.add)
                    else:
                        # h16 = LN2(l-1) without beta: z = (ps + b2[l-1]) + h16
                        b2p = lnp_t[:, (4 * (l - 1) + 3) * KC + ot:
                                    (4 * (l - 1) + 3) * KC + ot + 1]
                        nc.vector.scalar_tensor_tensor(
                            z_t[:, ot, :], ps[:], b2p, h16_t[:, ot, :],
                            ALU.add, ALU.add)

            # ---------- LN1 ----------
            ao16 = ao16p.tile([P, KC, S], BF16, tag="ao16")
            _layernorm(nc, tc, z_t, ao16, ln1g_t, ln1b_t, ones_col,
                       ones_row, z2p, smp)

            # ---------- GLU + wo (fused) ----------
            with tc.tile_pool(name="glu_ps", bufs=1, space="PSUM") as glu_ps, \
                 tc.tile_pool(name="wo_ps", bufs=6, space="PSUM") as wo_ps:
                bwo_t = bp.tile([1, KC, P], BF16, tag="bwo")
                nc.sync.dma_start(bwo_t[:], wslice(l, O_BWO, KC * P, (1, KC, P)))

                wo_acc = [wo_ps.tile([P, S], F32, tag="woacc", name=f"woacc{i}")
                          for i in range(KC)]
                for gt in range(NIC):
                    ps_g = glu_ps.tile([P, S], F32, tag="gps")
                    ps_u = glu_ps.tile([P, S], F32, tag="ups")
                    gw = wgp.tile([P, KC, 256], BF16, tag="gw", name=f"gw{gt}")
                    nc.sync.dma_start(
                        gw[:], wslice(l, O_GLU + gt * P * KC * 256, P * KC * 256,
                                      (P, KC, 256)))
                    for kc in range(KC):
                        nc.tensor.matmul(ps_g[:], gw[:, kc, 0:128], ao16[:, kc, :],
                                         start=(kc == 0), stop=(kc == KC - 1))
                    for kc in range(KC):
                        nc.tensor.matmul(ps_u[:], gw[:, kc, 128:256], ao16[:, kc, :],
                                         start=(kc == 0), stop=(kc == KC - 1))
                    xg = xgp.tile([P, S], BF16, tag="xg")
                    # gelu bias = glu_w[:INTER] @ ln1_b (deferred LN1 beta)
                    _i = nc.scalar.activation(
                        xg[:], ps_g[:], AF.Gelu,
                        bias=gb_t[:, 2 * l * NIC + gt:2 * l * NIC + gt + 1],
                        scale=1.0)
                    if gt == 0 and prev_exp[0] is not None:
                        add_dep_helper(_i.ins, prev_exp[0].ins, False,
                                       "act table grouping")
                    last_gelu[0] = _i
                    xc = xcp.tile([P, S], BF16, tag="xc")
                    # xc = (ps_u + bu)*xg, bu = glu_w[INTER:] @ ln1_b
                    nc.vector.scalar_tensor_tensor(
                        xc[:], ps_u[:],
                        gb_t[:, (2 * l + 1) * NIC + gt:(2 * l + 1) * NIC + gt + 1],
                        xg[:], ALU.add, ALU.mult)
                    wot_t = wop.tile([P, HID], BF16, tag="wot")
                    nc.sync.dma_start(
                        wot_t[:], wslice(l, O_OT + gt * P * HID, P * HID, (P, HID)))
                    for ot in range(KC):
                        nc.tensor.matmul(wo_acc[ot][:], wot_t[:, ot * P:(ot + 1) * P],
                                         xc[:], start=(gt == 0), stop=False)
                z2_t = zp.tile([P, KC, S], F32R, tag="z", name="z_mlp")
                for ot in range(KC):
                    nc.tensor.matmul(wo_acc[ot][:], bwo_t[:, ot, :], ones_s[:],
                                     start=False, stop=True)
                    # ao16 = LN1 without beta: z2 = (wo + b1) + ao16
                    b1p = lnp_t[:, (4 * l + 1) * KC + ot:
                                (4 * l + 1) * KC + ot + 1]
                    nc.vector.scalar_tensor_tensor(
                        z2_t[:, ot, :], wo_acc[ot][:], b1p, ao16[:, ot, :],
                        ALU.add, ALU.add)

            # ---------- LN2 -> next h ----------
            if l + 1 < n_layers:
                h16_t = h16p.tile([P, KC, S], BF16, tag="h16",
                                  name=f"h16_{l + 1}")
                _layernorm(nc, tc, z2_t, h16_t, ln2g_t, ln2b_t, ones_col,
                           ones_row, z2p, smp, ones_s, last_gelu[0])
            else:
                h_t = hpool.tile([P, KC, S], F32R, tag="h", name="h_last")
                _layernorm(nc, tc, z2_t, h_t, ln2g_t, ln2b_t, ones_col,
                           ones_row, z2p, smp, ones_s, last_gelu[0])

        # ---------- final mask + store ----------
        out_sb = zp.tile([P, KC, S], F32, tag="z", name="out_sb")
        if n_layers == 0:
            h_t = hpool.tile([P, KC, S], F32R, tag="h", name="h_last")
            hoff = NT + S + 1 + 4 * L * KC + GB_W + P
            nc.sync.dma_start(
                h_t[:], c32_d[:, hoff:hoff + KC * S].rearrange(
                    "p (c t) -> p c t", c=KC, t=S).bitcast(F32R))
        out_view = out_d[:].rearrange("(c p) t -> p c t", p=P)
        for c in range(KC):
            if n_layers == 0:
                nc.vector.tensor_tensor(out_sb[:, c, :],
                                        h_t[:, c, :].bitcast(F32),
                                        maskb_t[:], ALU.mult)
            else:
                # h = LN2(last) without beta: out = (h + b2)*mask
                b2p = lnp_t[:, (4 * (n_layers - 1) + 3) * KC + c:
                            (4 * (n_layers - 1) + 3) * KC + c + 1]
                nc.vector.scalar_tensor_tensor(
                    out_sb[:, c, :], h_t[:, c, :].bitcast(F32), b2p,
                    maskb_t[:], ALU.add, ALU.mult)
            nc.sync.dma_start(out_view[:, c, :], out_sb[:, c, :])

        stack.close()
        lp.__exit__(None, None, None)

    nc.finalize()
    return nc


def _prep_inputs(hidden_states, attention_mask, Wqkv_w, Wqkv_b, attn_out_w,
                 attn_out_b, ln1_g, ln1_b, glu_w, wo_w, wo_b, ln2_g, ln2_b,
                 r1, r2, r3):
    """Host-side sharding + weight layout transforms (shared across cores)."""
    f32 = np.float32
    shared = {}

    # ekb: exp(kerple bias) per (layer, head), Toeplitz [S, S] -> [P, NT*S]
    c1 = np.clip(r1.reshape(L, NH).astype(np.float64), 1e-7, None)
    c2 = np.clip(r2.reshape(L, NH).astype(np.float64), 1e-7, None)
    c3 = np.clip(r3.reshape(L, NH).astype(np.float64), 1e-7, None)
    idx = np.arange(S)
    rel = np.abs(idx[None, :] - idx[:, None]).astype(np.float64)  # [j, i]
    ekb = np.empty((L, NH, P, NT, S), f32)
    for l in range(L):
        for h in range(NH):
            relp = np.where(rel > 0, rel, 1.0) ** c3[l, h]
            relp = np.where(rel > 0, relp, 0.0)
            m = np.exp(-c1[l, h] * np.log1p(c2[l, h] * relp))  # [j, i]
            # [j, i] -> [jt, p, i] -> [p, jt, i]
            ekb[l, h] = m.reshape(NT, P, S).transpose(1, 0, 2).astype(f32)

    # LN outputs are stored WITHOUT beta on-device; fold each layer's
    # consumed beta into the next linear layer's bias: for layer l>=1 the
    # QKV input is LN2(l-1) missing ln2_b[l-1].
    Wqkv_b = np.asarray(Wqkv_b, f32).copy()
    for l in range(1, L):
        Wqkv_b[l] = Wqkv_b[l] + np.asarray(Wqkv_w[l], f32) @ np.asarray(
            ln2_b[l - 1], f32)

    wq = Wqkv_w[:, :HID, :] / 8.0           # fold 1/sqrt(DH)
    wk = Wqkv_w[:, HID:2 * HID, :]
    bq = Wqkv_b[:, :HID] / 8.0
    bk = Wqkv_b[:, HID:2 * HID]
    wqk = np.concatenate([wq, wk], axis=1)  # [L, 1536, HID]
    wqkT = np.transpose(wqk, (0, 2, 1))     # [L, HID, 1536]
    wqk_p = np.ascontiguousarray(
        wqkT.reshape(L, KC, P, 2 * KC, P).transpose(0, 3, 2, 1, 4))
    bqk_p = np.concatenate([bq, bk], axis=1)  # [L, 1536]

    wv = Wqkv_w[:, 2 * HID:, :]             # [L, 768v, 768]
    bv = Wqkv_b[:, 2 * HID:]
    wva = np.zeros((L, HID, NH * 65), f32)
    bva_p = np.zeros((L, NH * 65), f32)
    for h in range(NH):
        wva[:, :, 65 * h:65 * h + 64] = np.transpose(
            wv[:, DH * h:DH * (h + 1), :], (0, 2, 1))
        bva_p[:, 65 * h:65 * h + 64] = bv[:, DH * h:DH * (h + 1)]
        bva_p[:, 65 * h + 64] = 1.0
    wva_p = np.ascontiguousarray(
        wva.reshape(L, KC, P, 2, HALF).transpose(0, 3, 2, 1, 4))

    woaT = np.transpose(attn_out_w, (0, 2, 1))  # [L, HID, HID]
    woa_p = np.ascontiguousarray(
        woaT.reshape(L, KC, P, KC, P).transpose(0, 3, 2, 1, 4))

    glu = np.empty((L, HID, NIC, 256), f32)
    gw = np.transpose(glu_w, (0, 2, 1))     # [L, HID, 6144]
    for gt in range(NIC):
        glu[:, :, gt, 0:128] = gw[:, :, gt * P:(gt + 1) * P]
        glu[:, :, gt, 128:256] = gw[:, :, INTER + gt * P:INTER + (gt + 1) * P]
    glu_p = np.ascontiguousarray(
        glu.reshape(L, KC, P, NIC, 256).transpose(0, 3, 2, 1, 4))
    wot_p = np.ascontiguousarray(np.transpose(wo_w, (0, 2, 1)))  # [L, INTER, HID]

    w16 = np.concatenate([
        wqk_p.reshape(L, -1), wva_p.reshape(L, -1), woa_p.reshape(L, -1),
        glu_p.reshape(L, -1), wot_p.reshape(L, -1),
        bqk_p.reshape(L, -1), bva_p.reshape(L, -1),
        attn_out_b.reshape(L, -1), wo_b.reshape(L, -1),
    ], axis=1).astype(NPBF16)
    w16_head = np.concatenate([w16.reshape(-1), np.ones(S, NPBF16)])

    def pcol(v):  # [L, 768] -> [L, P, KC]
        return np.ascontiguousarray(v.reshape(L, KC, P).transpose(0, 2, 1)).astype(f32)

    # gains scaled by HID: the kernel computes rstd/HID (see _layernorm)
    lnp = np.stack([pcol(ln1_g * HID), pcol(ln1_b), pcol(ln2_g * HID),
                    pcol(ln2_b)], axis=1)  # [L, 4, P, KC]
    lnp = np.ascontiguousarray(lnp.transpose(2, 0, 1, 3)).reshape(P, 4 * L * KC)
    # deferred-LN1-beta glu biases: bg = glu_w[:,:INTER]... wait: glu rows are
    # outputs; bg[l] = glu_w[l, :INTER] @ ln1_b[l], bu[l] = glu_w[l, INTER:] @ b
    gbias = np.einsum("lof,lf->lo", np.asarray(glu_w, f32),
                      np.asarray(ln1_b, f32))      # [L, 2*INTER]
    bg = gbias[:, :INTER].reshape(L, NIC, P)       # [L, gt, p]
    bu = gbias[:, INTER:].reshape(L, NIC, P)
    gb = np.concatenate([bg, bu], axis=1)          # [L, 2*NIC, P]
    gb = np.ascontiguousarray(gb.transpose(2, 0, 1)).reshape(P, 2 * L * NIC)

    n_layers = _prep_inputs._n_layers
    in_maps = []
    for b in range(B):
        m = dict(shared)
        hTb = np.ascontiguousarray(hidden_states[b].T).astype(f32)
        m["h16"] = hTb.astype(NPBF16)
        mask = attention_mask[b].astype(f32)          # [S]
        # fold the key-side padding mask into the kerple table: softmax
        # numerator is exp(s)*ekb*mask_j, exactly 0 at padded keys
        mfold = mask.reshape(NT, P).T                 # [P, NT]
        ekb_b = (ekb * mfold[None, None, :, :, None]).astype(NPBF16)
        m["w16"] = np.ascontiguousarray(
            np.concatenate([w16_head, ekb_b.reshape(-1)]))
        mbias = (1.0 - mask) * -10000.0
        gbw = 2 * L * NIC
        base = NT + S + 1 + 4 * L * KC + gbw
        c32 = np.zeros((P, base + P + (KC * S if n_layers == 0 else 0)), f32)
        c32[:, 0:NT] = mbias.reshape(NT, P).T
        c32[:, NT:NT + S] = mask[None, :]
        c32[:, NT + S] = 1.0                      # ones_col
        c32[:, NT + S + 1:base - gbw] = lnp
        c32[:, base - gbw:base] = gb
        c32[0, base:base + P] = 1.0               # ones_row
        if n_layers == 0:
            c32[:, base + P:] = hTb.reshape(KC, P, S).transpose(1, 0, 2
                                                               ).reshape(P, KC * S)
        m["c32"] = c32
        in_maps.append(m)
    return in_maps


_prep_inputs._n_layers = L


def kernel(**inputs) -> np.ndarray:
    n_layers = int(inputs.pop("_n_layers", L))
    if n_layers not in _BUILT:
        _BUILT[n_layers] = _build(n_layers)
    nc = _BUILT[n_layers]
    _prep_inputs._n_layers = n_layers
    in_maps = _prep_inputs(**inputs)
    res = run_bass_kernel_spmd(nc, in_maps, list(range(B))).results
    out = np.empty((B, S, HID), np.float32)
    for b in range(B):
        out[b] = res[b]["out"].T
    return out
